# revision 24
# baseline (speedup 1.0000x reference)
"""TRN2 Bass kernel: fused LSTM cell (nn_CustomLSTMCell), 8-core tensor-parallel.

Strategy
--------
gates = x @ W_ih.T + b_ih + h_prev @ W_hh.T + b_hh  is computed as ONE GEMM
with contraction K = I + H = 4096 over xh = [x | h_prev] and W = [W_ih | W_hh].

The 4H gate dimension is tensor-parallel sharded across the 8 cores: core c
owns h-columns [c*256, (c+1)*256) of every gate (i, f, g, o).  Each core
computes gatesT [1024, 2048] = Wc @ xh.T with gate rows on partitions, so the
per-gate bias is a native per-partition scalar in scalar.activation, which
also applies sigmoid/tanh while evicting PSUM -> SBUF.  The LSTM cell update
(new_C = f*C + i*g, new_h = o*tanh(new_C)) runs on the vector engine, fully
overlapped with the tensor engine.  No collectives: output slices are
disjoint and gathered on the host.

Matmul operands are cast to fp16 on the host (halves DMA traffic, 4x PE rate
vs fp32, ~8x more mantissa than bf16); accumulation stays fp32 in PSUM and
the epilogue is fp32.
"""

import numpy as np
import ml_dtypes

B = 2048           # batch
I_DIM = 2048       # input features
H = 2048           # hidden
NCORES = 8
S = H // NCORES    # 256: per-core h-slice (per gate)
M_PER_CORE = 4 * S # 1024 gate rows per core
K = I_DIM + H      # 4096 fused contraction dim
P = 128
KC = K // P        # 32 contraction chunks
NT = B // 512      # 4 batch tiles of 512

_BF16 = np.float16

_CACHE = {}


def _build_program():
    from contextlib import ExitStack

    import concourse.mybir as mybir
    import concourse.tile as tile
    from concourse import bacc

    f32 = mybir.dt.float32
    bf16 = mybir.dt.float16
    AF = mybir.ActivationFunctionType

    nc = bacc.Bacc("TRN2", target_bir_lowering=False, debug=False)

    w_t = nc.dram_tensor("w_t", [K, M_PER_CORE], bf16, kind="ExternalInput").ap()
    xh_t = nc.dram_tensor("xh_t", [K, B], bf16, kind="ExternalInput").ap()
    # bias pre-shaped [128, 8] on the host: one clean 2D DMA (32B/partition)
    bias_d = nc.dram_tensor("bias", [P, 8], f32, kind="ExternalInput").ap()
    c_t = nc.dram_tensor("c_t", [S, B], f32, kind="ExternalInput").ap()
    outs = {
        name: nc.dram_tensor(name, [S, B], f32, kind="ExternalOutput").ap()
        for name in ("h_t", "cn_t", "f_t", "i_t", "g_t", "o_t")
    }

    # DRAM views with the 128-partition dim innermost on rows.
    w_r = w_t.rearrange("(a p) m -> p a m", p=P)        # [128, 32, 1024]
    xh_r = xh_t.rearrange("(a p) n -> p a n", p=P)      # [128, 32, 2048]
    c_r = c_t.rearrange("(a p) n -> p a n", p=P)        # [128, 2, 2048]

    HB = S // P  # 2 h-blocks of 128 per core
    # gate order within the per-core M dim: m-tile = hb*4 + gate (i,f,g,o)
    ACT_FN = [AF.Sigmoid, AF.Sigmoid, AF.Tanh, AF.Sigmoid]

    with tile.TileContext(nc) as tc, ExitStack() as ctx:
        w_pool = ctx.enter_context(tc.tile_pool(name="w", bufs=1))
        xh_pool = ctx.enter_context(tc.tile_pool(name="xh", bufs=2))
        c_pool = ctx.enter_context(tc.tile_pool(name="c", bufs=1))
        b_pool = ctx.enter_context(tc.tile_pool(name="b", bufs=1))
        psum_pool = ctx.enter_context(tc.tile_pool(name="ps", bufs=2, space="PSUM"))
        act_pool = ctx.enter_context(tc.tile_pool(name="act", bufs=3))

        # DMA dispatch costs ~0.6us each on one sequencer, so split the input
        # stream across the sync+gpsimd queues in consumption order, and keep
        # output stores off those queues (round-robin on all engines).  Each
        # matmul depends only on its own chunk tile's DMA, so the PE starts
        # as soon as chunk 0 lands and chases the stream.
        in_eng = [nc.sync, nc.gpsimd]
        _in_rr = [0]

        def in_dma(dst, src):
            in_eng[_in_rr[0] % 2].dma_start(dst, src)
            _in_rr[0] += 1

        # Preamble streams rotate over all three DMA-capable engines: one
        # engine's HWDGE/SWDGE queue sustains only ~116GB/s, and the first
        # two matmul groups consume ~218GB/s.  Scalar is idle until the
        # first epilogue (~38us), well after its share of the preamble.
        in3_eng = [nc.sync, nc.gpsimd, nc.scalar]
        _in3_rr = [0]

        def in3_dma(dst, src):
            in3_eng[_in3_rr[0] % 3].dma_start(dst, src)
            _in3_rr[0] += 1

        # Output stores all go on the Scalar queue (HWDGE): Sync/GpSimd stay
        # pure input streams in consumption order, and Scalar's in-order
        # queue naturally interleaves evictions with their stores.
        def out_dma(dst, src):
            nc.scalar.dma_start(dst, src)

        # Load bias first: tiny, and the first epilogue needs it.
        bias_all = b_pool.tile([P, 4 * HB], f32)
        nc.sync.dma_start(bias_all[:], bias_d[:, :])

        # Warmup matmuls on dummy data serve two purposes: HAM un-throttles
        # (~3.4us of sustained PE activity) AND the input DMA stream builds
        # a ~12us head start over PE consumption — the first two groups'
        # 12MB can only stream at ~200GB/s while the PE burns 218GB/s, so
        # starting the real matmuls early just trades warmup for mid-group
        # stalls and HAM re-throttles (measured worse).  Never read.
        dummy = b_pool.tile([P, 512], bf16)
        nc.vector.memset(dummy[:], 0.0)
        warm_ps = psum_pool.tile([P, 512], f32, name="ps0")
        NWARM = 48
        for i in range(NWARM):
            nc.tensor.matmul(
                warm_ps[:], dummy[:, 0:P], dummy[:],
                start=(i == 0), stop=(i == NWARM - 1),
            )

        # w chunks split into per-hb halves: the first matmul group only
        # needs the lo half (8MB with xh0 instead of 12MB before it can
        # finish).
        def alloc_xh(n):
            return [
                xh_pool.tile([P, 512], bf16, name=f"xh{k}") for k in range(KC)
            ]

        w_chunks = [
            [
                w_pool.tile([P, 4 * P], bf16, name=f"w{k}h{hb}")
                for hb in range(HB)
            ]
            for k in range(KC)
        ]
        xh_tiles = {0: alloc_xh(0), 1: alloc_xh(1)}
        for k in range(KC):
            in3_dma(w_chunks[k][0][:], w_r[:, k, 0 : 4 * P])
            in3_dma(xh_tiles[0][k][:], xh_r[:, k, 0:512])
        # C slice is only needed by the first epilogue (~38us in); keep it
        # behind the first-group stream so it can't delay chunk 0.
        c_all = c_pool.tile([P, HB, B], f32)
        for hb in range(HB):
            nc.gpsimd.dma_start(c_all[:, hb, :], c_r[:, hb, :])
        for k in range(KC):
            in3_dma(w_chunks[k][1][:], w_r[:, k, 4 * P : 8 * P])
            in3_dma(xh_tiles[1][k][:], xh_r[:, k, 512:1024])

        for n in range(NT):
            ns = slice(n * 512, (n + 1) * 512)
            # prefetch the n+1 tile one iteration ahead (slot frees when the
            # n-1 groups finish reading their generation)
            if n >= 1 and n + 1 < NT:
                xh_tiles[n + 1] = alloc_xh(n + 1)
                for k in range(KC):
                    in_dma(
                        xh_tiles[n + 1][k][:],
                        xh_r[:, k, (n + 1) * 512 : (n + 2) * 512],
                    )
            xh = xh_tiles[n]

            for hb in range(HB):
                final = n == NT - 1 and hb == HB - 1
                ps = [
                    psum_pool.tile([P, 512], f32, name=f"ps{g}") for g in range(4)
                ]
                if final:
                    # gate-major (f,i,g,o): each gate's PSUM closes early so
                    # the epilogue chain after the very last matmul is just
                    # o-sigmoid -> h-mul -> store.
                    for g in (1, 0, 2, 3):
                        for k in range(KC):
                            nc.tensor.matmul(
                                ps[g][:],
                                w_chunks[k][hb][:, g * P : (g + 1) * P],
                                xh[k][:],
                                start=(k == 0),
                                stop=(k == KC - 1),
                            )
                else:
                    for k in range(KC):
                        for g in range(4):
                            nc.tensor.matmul(
                                ps[g][:],
                                w_chunks[k][hb][:, g * P : (g + 1) * P],
                                xh[k][:],
                                start=(k == 0),
                                stop=(k == KC - 1),
                            )

                # epilogue: activations (+bias) evict PSUM, then cell update.
                # For the final group: f,i,g evict first and the cell-state
                # chain (fc/ig/cn/tanh) completes during the o-gate matmul
                # block, so only o-sigmoid + h-mul remain after the last MM.
                def gate_act(g):
                    m = hb * 4 + g
                    a = act_pool.tile([P, 512], f32, name=f"a{g}")
                    nc.scalar.activation(
                        a[:], ps[g][:], ACT_FN[g], bias=bias_all[:, m : m + 1]
                    )
                    return a

                acts = [None] * 4
                for g in (1, 0, 2) if final else (0, 1, 2, 3):
                    acts[g] = gate_act(g)
                fc = act_pool.tile([P, 512], f32, name="fc")
                nc.vector.tensor_mul(fc[:], acts[1][:], c_all[:, hb, ns])
                ig = act_pool.tile([P, 512], f32, name="ig")
                nc.vector.tensor_mul(ig[:], acts[0][:], acts[2][:])
                cn = act_pool.tile([P, 512], f32, name="cn")
                nc.vector.tensor_add(cn[:], ig[:], fc[:])
                th = act_pool.tile([P, 512], f32, name="th")
                nc.scalar.activation(th[:], cn[:], AF.Tanh)
                if final:
                    acts[3] = gate_act(3)
                ia, fa, ga, oa = acts
                hn = act_pool.tile([P, 512], f32, name="hn")
                nc.vector.tensor_mul(hn[:], oa[:], th[:])

                rs = slice(hb * P, (hb + 1) * P)
                pairs = (
                    ("f_t", fa),
                    ("i_t", ia),
                    ("g_t", ga),
                    ("o_t", oa),
                    ("cn_t", cn),
                    ("h_t", hn),
                )
                if final:
                    # split the tail stores into halves over all DMA queues
                    # so the post-compute drain is short
                    engs = [nc.scalar, nc.sync, nc.gpsimd]
                    j = 0
                    for name, t in pairs:
                        for half in range(2):
                            cs = slice(n * 512 + half * 256, n * 512 + (half + 1) * 256)
                            engs[j % 3].dma_start(
                                outs[name][rs, cs], t[:, half * 256 : (half + 1) * 256]
                            )
                            j += 1
                else:
                    for name, t in pairs:
                        out_dma(outs[name][rs, ns], t[:])

    nc.compile()
    return nc


def _get_program():
    if "nc" not in _CACHE:
        _CACHE["nc"] = _build_program()
    return _CACHE["nc"]


def _gate_row_index(core: int) -> np.ndarray:
    """Global rows of W/b (4H-dim) owned by `core`, in m-tile order."""
    idx = []
    for hb in range(S // P):
        for g in range(4):
            base = g * H + core * S + hb * P
            idx.extend(range(base, base + P))
    return np.asarray(idx)


def kernel(x, h_prev, C_prev, W_ih, b_ih, W_hh, b_hh):
    from concourse.bass_utils import run_bass_kernel_spmd

    nc = _get_program()

    xh_t = np.ascontiguousarray(
        np.concatenate([x, h_prev], axis=1).T
    ).astype(_BF16)  # [4096, 2048], shared by all cores
    bias_full = (b_ih + b_hh).astype(np.float32)

    in_maps = []
    for c in range(NCORES):
        idx = _gate_row_index(c)
        w_cat = np.concatenate([W_ih[idx], W_hh[idx]], axis=1)  # [1024, 4096]
        in_maps.append(
            {
                "w_t": np.ascontiguousarray(w_cat.T).astype(_BF16),
                "xh_t": xh_t,
                "bias": np.ascontiguousarray(bias_full[idx].reshape(8, P).T),
                "c_t": np.ascontiguousarray(C_prev[:, c * S : (c + 1) * S].T),
            }
        )

    _CACHE["last_in_maps"] = in_maps
    res = run_bass_kernel_spmd(nc, in_maps, core_ids=list(range(NCORES)))

    def gather(name):
        t = np.concatenate([res.results[c][name] for c in range(NCORES)], axis=0)
        return np.ascontiguousarray(t.T)  # [B, H]

    return (
        gather("h_t"),
        gather("cn_t"),
        gather("f_t"),
        gather("i_t"),
        gather("g_t"),
        gather("o_t"),
    )


# revision 25
# speedup vs baseline: 1.0414x; 1.0414x over previous
"""TRN2 Bass kernel: fused LSTM cell (nn_CustomLSTMCell), 8-core tensor-parallel.

Strategy
--------
gates = x @ W_ih.T + b_ih + h_prev @ W_hh.T + b_hh  is computed as ONE GEMM
with contraction K = I + H = 4096 over xh = [x | h_prev] and W = [W_ih | W_hh].

The 4H gate dimension is tensor-parallel sharded across the 8 cores: core c
owns h-columns [c*256, (c+1)*256) of every gate (i, f, g, o).  Each core
computes gatesT [1024, 2048] = Wc @ xh.T with gate rows on partitions, so the
per-gate bias is a native per-partition scalar in scalar.activation, which
also applies sigmoid/tanh while evicting PSUM -> SBUF.  The LSTM cell update
(new_C = f*C + i*g, new_h = o*tanh(new_C)) runs on the vector engine, fully
overlapped with the tensor engine.  No collectives: output slices are
disjoint and gathered on the host.

Matmul operands are cast to fp16 on the host (halves DMA traffic, 4x PE rate
vs fp32, ~8x more mantissa than bf16); accumulation stays fp32 in PSUM and
the epilogue is fp32.
"""

import numpy as np

B = 2048           # batch
I_DIM = 2048       # input features
H = 2048           # hidden
NCORES = 8
S = H // NCORES    # 256: per-core h-slice (per gate)
M_PER_CORE = 4 * S # 1024 gate rows per core
K = I_DIM + H      # 4096 fused contraction dim
P = 128
KC = K // P        # 32 contraction chunks
NT = B // 512      # 4 batch tiles of 512

_BF16 = np.float16

_CACHE = {}


def _build_program():
    from contextlib import ExitStack

    import concourse.mybir as mybir
    import concourse.tile as tile
    from concourse import bacc

    f32 = mybir.dt.float32
    bf16 = mybir.dt.float16
    AF = mybir.ActivationFunctionType

    nc = bacc.Bacc("TRN2", target_bir_lowering=False, debug=False)

    w_t = nc.dram_tensor("w_t", [K, M_PER_CORE], bf16, kind="ExternalInput").ap()
    xh_t = nc.dram_tensor("xh_t", [K, B], bf16, kind="ExternalInput").ap()
    # bias pre-shaped [128, 8] on the host: one clean 2D DMA (32B/partition)
    bias_d = nc.dram_tensor("bias", [P, 8], f32, kind="ExternalInput").ap()
    c_t = nc.dram_tensor("c_t", [S, B], f32, kind="ExternalInput").ap()
    outs = {
        name: nc.dram_tensor(name, [S, B], f32, kind="ExternalOutput").ap()
        for name in ("h_t", "cn_t", "f_t", "i_t", "g_t", "o_t")
    }

    # DRAM views with the 128-partition dim innermost on rows.
    w_r = w_t.rearrange("(a p) m -> p a m", p=P)        # [128, 32, 1024]
    xh_r = xh_t.rearrange("(a p) n -> p a n", p=P)      # [128, 32, 2048]
    c_r = c_t.rearrange("(a p) n -> p a n", p=P)        # [128, 2, 2048]

    HB = S // P  # 2 h-blocks of 128 per core
    # gate order within the per-core M dim: m-tile = hb*4 + gate (i,f,g,o)
    ACT_FN = [AF.Sigmoid, AF.Sigmoid, AF.Tanh, AF.Sigmoid]

    with tile.TileContext(nc) as tc, ExitStack() as ctx:
        w_pool = ctx.enter_context(tc.tile_pool(name="w", bufs=1))
        xh_pool = ctx.enter_context(tc.tile_pool(name="xh", bufs=2))
        c_pool = ctx.enter_context(tc.tile_pool(name="c", bufs=1))
        b_pool = ctx.enter_context(tc.tile_pool(name="b", bufs=1))
        psum_pool = ctx.enter_context(tc.tile_pool(name="ps", bufs=2, space="PSUM"))
        act_pool = ctx.enter_context(tc.tile_pool(name="act", bufs=3))

        # DMA dispatch costs ~0.6us each on one sequencer, so split the input
        # stream across the sync+gpsimd queues in consumption order, and keep
        # output stores off those queues (round-robin on all engines).  Each
        # matmul depends only on its own chunk tile's DMA, so the PE starts
        # as soon as chunk 0 lands and chases the stream.
        in_eng = [nc.sync, nc.gpsimd]
        _in_rr = [0]

        def in_dma(dst, src):
            in_eng[_in_rr[0] % 2].dma_start(dst, src)
            _in_rr[0] += 1

        # Preamble streams rotate over all three DMA-capable engines: one
        # engine's HWDGE/SWDGE queue sustains only ~116GB/s, and the first
        # two matmul groups consume ~218GB/s.  Scalar is idle until the
        # first epilogue (~38us), well after its share of the preamble.
        in3_eng = [nc.sync, nc.gpsimd, nc.scalar]
        _in3_rr = [0]

        def in3_dma(dst, src):
            in3_eng[_in3_rr[0] % 3].dma_start(dst, src)
            _in3_rr[0] += 1

        # Output stores all go on the Scalar queue (HWDGE): Sync/GpSimd stay
        # pure input streams in consumption order, and Scalar's in-order
        # queue naturally interleaves evictions with their stores.
        def out_dma(dst, src):
            nc.scalar.dma_start(dst, src)

        # Load bias first: tiny, and the first epilogue needs it.
        bias_all = b_pool.tile([P, 4 * HB], f32)
        nc.sync.dma_start(bias_all[:], bias_d[:, :])

        # A few matmuls on dummy data bridge the framework-preamble gap so
        # HAM warm-up overlaps the first chunk DMAs.  Never read.
        dummy = b_pool.tile([P, 512], bf16)
        nc.vector.memset(dummy[:], 0.0)
        warm_ps = psum_pool.tile([P, 512], f32, name="ps0")
        NWARM = 3
        for i in range(NWARM):
            nc.tensor.matmul(
                warm_ps[:], dummy[:, 0:P], dummy[:],
                start=(i == 0), stop=(i == NWARM - 1),
            )

        # w chunks split into per-hb halves: the first matmul group only
        # needs the lo half (8MB with xh0 instead of 12MB before it can
        # finish).
        def alloc_xh(n):
            return [
                xh_pool.tile([P, 512], bf16, name=f"xh{k}") for k in range(KC)
            ]

        w_chunks = [
            [
                w_pool.tile([P, 4 * P], bf16, name=f"w{k}h{hb}")
                for hb in range(HB)
            ]
            for k in range(KC)
        ]
        xh_tiles = {0: alloc_xh(0), 1: alloc_xh(1)}
        for k in range(KC):
            in3_dma(w_chunks[k][0][:], w_r[:, k, 0 : 4 * P])
            in3_dma(xh_tiles[0][k][:], xh_r[:, k, 0:512])
        # C slice is only needed by the first epilogue (~38us in); keep it
        # behind the first-group stream so it can't delay chunk 0.
        c_all = c_pool.tile([P, HB, B], f32)
        for hb in range(HB):
            nc.gpsimd.dma_start(c_all[:, hb, :], c_r[:, hb, :])
        for k in range(KC):
            in3_dma(w_chunks[k][1][:], w_r[:, k, 4 * P : 8 * P])
            in3_dma(xh_tiles[1][k][:], xh_r[:, k, 512:1024])

        for n in range(NT):
            ns = slice(n * 512, (n + 1) * 512)
            # prefetch the n+1 tile one iteration ahead (slot frees when the
            # n-1 groups finish reading their generation)
            if n >= 1 and n + 1 < NT:
                xh_tiles[n + 1] = alloc_xh(n + 1)
                for k in range(KC):
                    in_dma(
                        xh_tiles[n + 1][k][:],
                        xh_r[:, k, (n + 1) * 512 : (n + 2) * 512],
                    )
            xh = xh_tiles[n]

            for hb in range(HB):
                final = n == NT - 1 and hb == HB - 1
                ps = [
                    psum_pool.tile([P, 512], f32, name=f"ps{g}") for g in range(4)
                ]
                if final:
                    # gate-major (f,i,g,o): each gate's PSUM closes early so
                    # the epilogue chain after the very last matmul is just
                    # o-sigmoid -> h-mul -> store.
                    for g in (1, 0, 2, 3):
                        for k in range(KC):
                            nc.tensor.matmul(
                                ps[g][:],
                                w_chunks[k][hb][:, g * P : (g + 1) * P],
                                xh[k][:],
                                start=(k == 0),
                                stop=(k == KC - 1),
                            )
                else:
                    for k in range(KC):
                        for g in range(4):
                            nc.tensor.matmul(
                                ps[g][:],
                                w_chunks[k][hb][:, g * P : (g + 1) * P],
                                xh[k][:],
                                start=(k == 0),
                                stop=(k == KC - 1),
                            )

                # epilogue: activations (+bias) evict PSUM, then cell update.
                # For the final group: f,i,g evict first and the cell-state
                # chain (fc/ig/cn/tanh) completes during the o-gate matmul
                # block, so only o-sigmoid + h-mul remain after the last MM.
                def gate_act(g):
                    m = hb * 4 + g
                    a = act_pool.tile([P, 512], f32, name=f"a{g}")
                    nc.scalar.activation(
                        a[:], ps[g][:], ACT_FN[g], bias=bias_all[:, m : m + 1]
                    )
                    return a

                acts = [None] * 4
                for g in (1, 0, 2) if final else (0, 1, 2, 3):
                    acts[g] = gate_act(g)
                fc = act_pool.tile([P, 512], f32, name="fc")
                nc.vector.tensor_mul(fc[:], acts[1][:], c_all[:, hb, ns])
                ig = act_pool.tile([P, 512], f32, name="ig")
                nc.vector.tensor_mul(ig[:], acts[0][:], acts[2][:])
                cn = act_pool.tile([P, 512], f32, name="cn")
                nc.vector.tensor_add(cn[:], ig[:], fc[:])
                th = act_pool.tile([P, 512], f32, name="th")
                nc.scalar.activation(th[:], cn[:], AF.Tanh)
                if final:
                    acts[3] = gate_act(3)
                ia, fa, ga, oa = acts
                hn = act_pool.tile([P, 512], f32, name="hn")
                nc.vector.tensor_mul(hn[:], oa[:], th[:])

                rs = slice(hb * P, (hb + 1) * P)
                pairs = (
                    ("f_t", fa),
                    ("i_t", ia),
                    ("g_t", ga),
                    ("o_t", oa),
                    ("cn_t", cn),
                    ("h_t", hn),
                )
                if final:
                    # split the tail stores into halves over all DMA queues
                    # so the post-compute drain is short
                    engs = [nc.scalar, nc.sync, nc.gpsimd]
                    j = 0
                    for name, t in pairs:
                        for half in range(2):
                            cs = slice(n * 512 + half * 256, n * 512 + (half + 1) * 256)
                            engs[j % 3].dma_start(
                                outs[name][rs, cs], t[:, half * 256 : (half + 1) * 256]
                            )
                            j += 1
                else:
                    for name, t in pairs:
                        out_dma(outs[name][rs, ns], t[:])

    nc.compile()
    return nc


def _get_program():
    if "nc" not in _CACHE:
        _CACHE["nc"] = _build_program()
    return _CACHE["nc"]


def _gate_row_index(core: int) -> np.ndarray:
    """Global rows of W/b (4H-dim) owned by `core`, in m-tile order."""
    idx = []
    for hb in range(S // P):
        for g in range(4):
            base = g * H + core * S + hb * P
            idx.extend(range(base, base + P))
    return np.asarray(idx)


def kernel(x, h_prev, C_prev, W_ih, b_ih, W_hh, b_hh):
    from concourse.bass_utils import run_bass_kernel_spmd

    nc = _get_program()

    xh_t = np.ascontiguousarray(
        np.concatenate([x, h_prev], axis=1).T
    ).astype(_BF16)  # [4096, 2048], shared by all cores
    bias_full = (b_ih + b_hh).astype(np.float32)

    in_maps = []
    for c in range(NCORES):
        idx = _gate_row_index(c)
        w_cat = np.concatenate([W_ih[idx], W_hh[idx]], axis=1)  # [1024, 4096]
        in_maps.append(
            {
                "w_t": np.ascontiguousarray(w_cat.T).astype(_BF16),
                "xh_t": xh_t,
                "bias": np.ascontiguousarray(bias_full[idx].reshape(8, P).T),
                "c_t": np.ascontiguousarray(C_prev[:, c * S : (c + 1) * S].T),
            }
        )

    _CACHE["last_in_maps"] = in_maps
    res = run_bass_kernel_spmd(nc, in_maps, core_ids=list(range(NCORES)))

    def gather(name):
        t = np.concatenate([res.results[c][name] for c in range(NCORES)], axis=0)
        return np.ascontiguousarray(t.T)  # [B, H]

    return (
        gather("h_t"),
        gather("cn_t"),
        gather("f_t"),
        gather("i_t"),
        gather("g_t"),
        gather("o_t"),
    )
